# revision 5
# baseline (speedup 1.0000x reference)
"""TRN2 Bass kernel for nn_CosClassifier: sim = 10*scalar * cos_sim(inputs, proto).

Data-parallel over 8 NeuronCores: each core computes a (2048, 4096) slab of the
(16384, 4096) similarity matrix. The kernel is HBM-bound (38MB/core at
~358GB/s => ~106us floor), so the schedule keeps the 16 SDMA queues saturated:

  1. Input DMA order xsg0, psg0..7, xsg1..3 - the last chunks to land are
     x-slabs first needed at i=4 (~t=36us), so proto is resident by ~15us.
  2. Norms via ACT Square(+accum) -> ACT Rsqrt (Square/Rsqrt/Copy share one
     activation table: no reloads). Proto rows pre-scaled by 10*scalar/||p||
     on DVE; x stays raw - 1/||x|| is applied per-partition at the PSUM
     drain (tensor_scalar_mul), keeping x's critical path DMA->transpose.
  3. PE transposes 128x128 blocks into PSUM; DVE casts drain to f32r so the
     main matmul streams at 1 cycle/row (k-alternating lhsT; same-lhsT b2b
     fp32r is pathologically slow).
  4. Row-block i=0's matmuls interleave with proto processing (n-block g
     right after psg_g), so output DMA is queued from ~11us.
  5. Output rows land in full-width oq[128,4096] tiles -> one contiguous 2MB
     DMA per row-block (17 output DMAs total vs 64) to keep the Sync engine
     (~630ns HWDGE issue cost each) far from saturation.
"""
import sys

sys.path.insert(0, "/opt/trn_rl_repo")

import numpy as np

B, C, D = 16384, 4096, 256
NCORES = 8
BS = B // NCORES          # 2048 rows per core
NB = BS // 128            # 16 b-tiles per core
NCT = C // 128            # 32 c-tiles (proto rows)
NK = D // 128             # 2 k-tiles
NN = C // 512             # 8 n-blocks of 512
SGT = 4                   # tiles per subgroup (512KB)
XSG = NB // SGT           # 4 x subgroups
PSG = NCT // SGT          # 8 proto subgroups

_compiled = None


def _build():
    import concourse.bacc as bacc
    import concourse.mybir as mybir
    import concourse.tile as tile

    f32 = mybir.dt.float32
    f32r = mybir.dt.float32r
    Act = mybir.ActivationFunctionType

    nc = bacc.Bacc("TRN2", target_bir_lowering=False, debug=False,
                   num_devices=NCORES)

    x_d = nc.dram_tensor("x", [BS, D], f32, kind="ExternalInput").ap()
    p_d = nc.dram_tensor("proto", [C, D], f32, kind="ExternalInput").ap()
    s_d = nc.dram_tensor("scalar", [1, 1], f32, kind="ExternalInput").ap()
    id_d = nc.dram_tensor("identity", [128, 128], f32, kind="ExternalInput").ap()
    out_d = nc.dram_tensor("out", [BS, C], f32, kind="ExternalOutput").ap()

    with tile.TileContext(nc) as tc:
        with tc.tile_pool(name="sbuf", bufs=1) as pool, \
             tc.tile_pool(name="outp", bufs=3) as outp, \
             tc.tile_pool(name="psum_t", bufs=2, space="PSUM") as psum_t, \
             tc.tile_pool(name="psum_m", bufs=6, space="PSUM") as psum_m:

            x_r = x_d.rearrange("(n p) d -> p n d", p=128)       # [128, NB, 256]
            p_r = p_d.rearrange("(n p) d -> p n d", p=128)       # [128, NCT, 256]

            def load_x(g):
                t = pool.tile([128, SGT * D], f32, tag=f"xsg{g}")
                nc.sync.dma_start(
                    t[:].rearrange("p (n d) -> p n d", d=D),
                    x_r[:, g * SGT:(g + 1) * SGT, :])
                return t

            def load_p(g):
                t = pool.tile([128, SGT * D], f32, tag=f"psg{g}")
                nc.sync.dma_start(
                    t[:].rearrange("p (n d) -> p n d", d=D),
                    p_r[:, g * SGT:(g + 1) * SGT, :])
                return t

            ident = pool.tile([128, 128], f32, tag="ident")
            nc.sync.dma_start(ident[:], id_d[:, :])
            sc = pool.tile([1, 1], f32, tag="sc")
            nc.sync.dma_start(sc[:], s_d[:, :])
            sc_b = pool.tile([128, 1], f32, tag="sc_b")
            nc.gpsimd.partition_broadcast(sc_b[:], sc[:])

            xsg = {}
            psg = {}
            # x-slab 0 first (feeds b-tiles 0-3 / all of i=0..3), then the
            # whole proto bank (feeds every n-block of i=0), x rest last.
            xsg[0] = load_x(0)
            for g in range(PSG):
                psg[g] = load_p(g)
            for g in range(1, XSG):
                xsg[g] = load_x(g)

            # transposed operands (f32r)
            # xt: k-block k at cols k*BS, b-tile i at +i*128
            xt = pool.tile([128, NK * BS], f32r, tag="xt")
            # pt: k-block k at cols k*C, c-tile j at +j*128
            pt = pool.tile([128, NK * C], f32r, tag="pt")

            xssq = pool.tile([128, NB], f32, tag="xssq")
            xnrm = pool.tile([128, NB], f32, tag="xnrm")
            xinv = pool.tile([128, NB], f32, tag="xinv")   # 1/||x_b||

            def xproc(g):
                grp = xsg[g]
                # transposes don't wait on the norm chain (x stays raw)
                for k in range(NK):
                    tp = psum_t.tile([128, SGT * 128], f32, tag="tp")
                    for t in range(SGT):
                        nc.tensor.transpose(
                            tp[:, t * 128:(t + 1) * 128],
                            grp[:, t * D + k * 128: t * D + (k + 1) * 128],
                            ident[:])
                    nc.vector.tensor_copy(
                        xt[:, k * BS + g * SGT * 128:
                           k * BS + (g + 1) * SGT * 128], tp[:])
                for t in range(SGT):
                    scr = pool.tile([128, D], f32, tag=f"scr{t % 4}")
                    nc.scalar.activation(scr[:], grp[:, t * D:(t + 1) * D],
                                         Act.Square,
                                         accum_out=xssq[:, g * SGT + t:
                                                        g * SGT + t + 1])
                nc.scalar.activation(xnrm[:, g * SGT:(g + 1) * SGT],
                                     xssq[:, g * SGT:(g + 1) * SGT], Act.Sqrt)
                nc.vector.reciprocal(xinv[:, g * SGT:(g + 1) * SGT],
                                     xnrm[:, g * SGT:(g + 1) * SGT])

            def pproc(g):
                grp = psg[g]
                pssq = pool.tile([128, SGT], f32, tag=f"pssq{g % 2}")
                for t in range(SGT):
                    scr = pool.tile([128, D], f32, tag=f"scr{t % 4}")
                    nc.scalar.activation(scr[:], grp[:, t * D:(t + 1) * D],
                                         Act.Square,
                                         accum_out=pssq[:, t:t + 1])
                pnrm = pool.tile([128, SGT], f32, tag=f"pnrm{g % 2}")
                pscl = pool.tile([128, SGT], f32, tag=f"pscl{g % 2}")
                # sqrt(0.01*ssq) = ||p||/10; recip = 10/||p||; then *scalar
                nc.scalar.activation(pnrm[:], pssq[:], Act.Sqrt, scale=0.01)
                nc.vector.reciprocal(pscl[:], pnrm[:])
                nc.vector.tensor_scalar_mul(pscl[:], pscl[:], sc_b[:])
                for t in range(SGT):
                    nc.vector.tensor_scalar_mul(
                        grp[:, t * D:(t + 1) * D], grp[:, t * D:(t + 1) * D],
                        pscl[:, t:t + 1])
                for k in range(NK):
                    tp = psum_t.tile([128, SGT * 128], f32, tag="tp")
                    for t in range(SGT):
                        nc.tensor.transpose(
                            tp[:, t * 128:(t + 1) * 128],
                            grp[:, t * D + k * 128: t * D + (k + 1) * 128],
                            ident[:])
                    nc.vector.tensor_copy(
                        pt[:, k * C + g * SGT * 128:
                           k * C + (g + 1) * SGT * 128], tp[:])

            def mm_block(i, n, oq, drain_eng):
                ps = psum_m.tile([128, 512], f32, tag="mm")
                for k in range(NK):
                    nc.tensor.matmul(
                        ps[:],
                        xt[:, k * BS + i * 128: k * BS + (i + 1) * 128],
                        pt[:, k * C + n * 512: k * C + (n + 1) * 512],
                        start=(k == 0), stop=(k == NK - 1))
                dst = oq[:, n * 512:(n + 1) * 512]
                if drain_eng == "act":
                    nc.scalar.activation(dst, ps[:], Act.Copy,
                                         scale=xinv[:, i:i + 1])
                else:
                    nc.vector.tensor_scalar_mul(dst, ps[:], xinv[:, i:i + 1])

            # ---- i=0 interleaved with proto processing ----
            xproc(0)
            oq0 = outp.tile([128, C], f32, tag="oq")
            for g in range(PSG):
                pproc(g)
                mm_block(0, g, oq0, "dve")
                if g == SGT - 1:
                    nc.sync.dma_start(out_d[0:128, 0:SGT * 512],
                                      oq0[:, 0:SGT * 512])
            nc.sync.dma_start(out_d[0:128, SGT * 512:C], oq0[:, SGT * 512:C])

            # remaining x processing (lands ~16-19us; only needed from i=4)
            for g in range(1, XSG):
                xproc(g)

            # ---- main loop: one full output row-block per iteration ----
            for i in range(1, NB):
                oq = outp.tile([128, C], f32, tag="oq")
                for n in range(NN):
                    # keep 2 of 8 drains on ACT (idle after norm phase)
                    mm_block(i, n, oq, "act" if n % 4 == 3 else "dve")
                nc.sync.dma_start(out_d[i * 128:(i + 1) * 128, :], oq[:])

    nc.compile()
    return nc


def _get_compiled():
    global _compiled
    if _compiled is None:
        _compiled = _build()
    return _compiled


def kernel(inputs, proto, scalar, _trace=False, **_tr_kw):
    from concourse.bass_utils import run_bass_kernel_spmd

    nc = _get_compiled()
    inputs = np.ascontiguousarray(inputs, dtype=np.float32)
    proto = np.ascontiguousarray(proto, dtype=np.float32)
    sc = np.asarray(scalar, dtype=np.float32).reshape(1, 1)
    ident = np.eye(128, dtype=np.float32)

    in_maps = []
    for c in range(NCORES):
        in_maps.append({
            "x": inputs[c * BS:(c + 1) * BS],
            "proto": proto,
            "scalar": sc,
            "identity": ident,
        })
    res = run_bass_kernel_spmd(nc, in_maps, core_ids=list(range(NCORES)),
                               trace=_trace, **_tr_kw)
    out = np.concatenate([res.results[c]["out"] for c in range(NCORES)], axis=0)
    if _trace:
        kernel.last_results = res
    return out
